# revision 21
# baseline (speedup 1.0000x reference)
"""AdaptiveECE on 8 Trainium2 NeuronCores.

Data-parallel over N=1,000,000 rows: each core streams its 125,000-row shard
of logits [N,128] once (the memory-bound part) and reduces it to two small
per-row tensors:

  - s[r]  = sum_c exp(x[r,c])      (ScalarE big-op Exp, VectorE segmented
                                    reduce_sum)
  - mt[r] = max_c f32((x & ~0x7F) | (127-c))
                                   (VectorE fused scalar_tensor_tensor pack +
                                    segmented reduce_max)
    mt packs the row max (high bits) and the argmax index (low 7 mantissa
    bits) into one float: conf = exp(mt & ~0x7F) / s on device; argmax is
    decoded from the low bits on the host for the accuracy compare.

The host then finishes exactly as the reference: accuracy from argmax vs
labels, global sort of confidences, equal-count bin edges via interp,
searchsorted binning, per-bin (count, conf_sum, acc_sum), ECE.

Layout: each partition line holds G=8 consecutive rows (4KB contiguous DMA
runs). Output column (t*G + j), partition p  <->  shard row t*G*128 + p*G + j.
"""

from contextlib import ExitStack

import numpy as np

import concourse.bass as bass
import concourse.tile as tile
from concourse import bacc, mybir
from concourse.bass_utils import run_bass_kernel_spmd

N = 1_000_000
C = 128
NBINS = 15
NCORES = 8
ROWS = N // NCORES  # 125_000 per core
MASK = 0xFFFFFF80
G = 8  # rows per partition line (4KB contiguous DMA runs)
JA = 5  # of each 16 columns, this many row-sums go to ScalarE accum
CHUNK_T = 12  # t-groups (of G*128 rows) per chunk
PACK_ARGMAX = False  # False: mt = plain row max; acc via host label-gather

_CACHE: dict = {}
LAST_RESULT = None  # BassKernelResults of the most recent device run


def _build(rows: int, chunk_t: int = CHUNK_T):
    gr = G * 128  # rows per t-group
    tfull = rows // gr  # full t-groups
    tail = rows - tfull * gr  # leftover rows
    tail_p = tail // G  # tail partitions (tail must divide by G)
    assert tail % G == 0, (rows, tail)
    tt = (tfull + (1 if tail else 0)) * G  # output columns

    nc = bacc.Bacc("TRN2", target_bir_lowering=False, debug=False)
    lg = nc.dram_tensor("logits", [rows, C], mybir.dt.float32, kind="ExternalInput").ap()
    conf_d = nc.dram_tensor("conf", [128, tt], mybir.dt.float32, kind="ExternalOutput").ap()
    mt_d = nc.dram_tensor("mt", [128, tt], mybir.dt.float32, kind="ExternalOutput").ap()

    # [p, t, (j c)] view: row t*1024 + p*8 + j; (j c) is 4KB-contiguous per (p,t)
    lg_t = (
        lg[0 : tfull * gr, :].rearrange("(t p j) c -> p t (j c)", p=128, j=G)
        if tfull
        else None
    )

    with tile.TileContext(nc) as tc, ExitStack() as ctx:
        singles = ctx.enter_context(tc.tile_pool(name="singles", bufs=1))
        xpool = ctx.enter_context(tc.tile_pool(name="x", bufs=2))
        bpool = ctx.enter_context(tc.tile_pool(name="xb", bufs=2))
        epool = ctx.enter_context(tc.tile_pool(name="e", bufs=2))
        spool = ctx.enter_context(tc.tile_pool(name="scratch", bufs=2))

        pat = singles.tile([128, C], mybir.dt.uint32)
        nc.gpsimd.iota(pat[:], pattern=[[-1, C]], base=127, channel_multiplier=0)
        maskt = singles.tile([128, 1], mybir.dt.uint32)
        nc.vector.memset(maskt[:], MASK)

        sraw = singles.tile([128, tt], mybir.dt.float32)
        mt_sb = singles.tile([128, tt], mybir.dt.float32)
        mtr = singles.tile([128, tt], mybir.dt.float32)
        em = singles.tile([128, tt], mybir.dt.float32)
        rs = singles.tile([128, tt], mybir.dt.float32)
        conf_sb = singles.tile([128, tt], mybir.dt.float32)

        chunks = []
        t0 = 0
        while t0 < tfull:
            n = min(chunk_t, tfull - t0)
            chunks.append([t0, n, False])
            t0 += n
        if tail:
            if chunks and chunks[-1][1] < chunk_t:
                chunks[-1][2] = True
            else:
                chunks.append([tfull, 0, True])

        for t0, nfull, has_tail in chunks:
            nt = nfull + (1 if has_tail else 0)
            ncols = nt * G  # output columns this chunk
            x = xpool.tile([128, ncols, C], mybir.dt.float32)
            if nfull:
                nc.sync.dma_start(
                    x[:, 0 : nfull * G, :].rearrange("p a c -> p (a c)").rearrange(
                        "p (t b) -> p t b", b=G * C
                    ),
                    lg_t[:, t0 : t0 + nfull, :],
                )
            if has_tail:
                nc.vector.memset(x[:, nfull * G :, :], 0.0)
                tail_src = lg[tfull * gr : rows, :].rearrange("(p j) c -> p (j c)", j=G)
                nc.sync.dma_start(
                    x[0:tail_p, nfull * G :, :].rearrange("p a c -> p (a c)"), tail_src
                )

            if PACK_ARGMAX:
                xu = x[:].bitcast(mybir.dt.uint32)
                xb = bpool.tile([128, ncols, C], mybir.dt.uint32)
                pat_ap = pat[:]
                pat_bc = bass.AP(
                    tensor=pat_ap.tensor, offset=pat_ap.offset,
                    ap=[list(pat_ap.ap[0]), [0, ncols], list(pat_ap.ap[1])],
                )
                # xb = (x & ~0x7F) | (127-c): one fused DVE pass
                nc.vector.scalar_tensor_tensor(
                    xb[:], xu, maskt[:], pat_bc,
                    op0=mybir.AluOpType.bitwise_and, op1=mybir.AluOpType.bitwise_or,
                )
                nc.vector.reduce_max(
                    mt_sb[:, t0 * G : t0 * G + ncols],
                    xb[:].bitcast(mybir.dt.float32),
                    axis=mybir.AxisListType.X,
                )
            else:
                nc.vector.reduce_max(
                    mt_sb[:, t0 * G : t0 * G + ncols], x[:],
                    axis=mybir.AxisListType.X,
                )
            # row sums of exp(x): within each chunk, the first ka columns go
            # through ScalarE per-tile exp+accum, the rest through one big
            # ScalarE exp + VectorE segmented reduce — balances both engines
            # with only contiguous APs.
            ka = (ncols * JA) // 16
            for col in range(ka):
                scr = spool.tile([128, C], mybir.dt.float32)
                nc.scalar.activation(
                    scr[:], x[:, col, :], mybir.ActivationFunctionType.Exp,
                    accum_out=sraw[:, t0 * G + col : t0 * G + col + 1],
                )
            if ka < ncols:
                e = epool.tile([128, ncols - ka, C], mybir.dt.float32)
                nc.scalar.activation(
                    e[:], x[:, ka:ncols, :], mybir.ActivationFunctionType.Exp
                )
                nc.vector.reduce_sum(
                    sraw[:, t0 * G + ka : t0 * G + ncols], e[:],
                    axis=mybir.AxisListType.X,
                )

        if PACK_ARGMAX:
            nc.vector.tensor_scalar(
                mtr[:].bitcast(mybir.dt.uint32), mt_sb[:].bitcast(mybir.dt.uint32),
                scalar1=MASK, scalar2=None, op0=mybir.AluOpType.bitwise_and,
            )
            nc.scalar.activation(em[:], mtr[:], mybir.ActivationFunctionType.Exp)
        else:
            nc.scalar.activation(em[:], mt_sb[:], mybir.ActivationFunctionType.Exp)
        nc.vector.reciprocal(rs[:], sraw[:])
        nc.vector.tensor_tensor(conf_sb[:], em[:], rs[:], op=mybir.AluOpType.mult)
        nc.sync.dma_start(conf_d, conf_sb[:])
        nc.sync.dma_start(mt_d, mt_sb[:])

    nc.compile()
    return nc


def _decode(conf_2d, mt_2d, rows):
    """Device outputs [128, TT] -> per-row conf [rows], argmax [rows].

    Column t*G+j, partition p <-> row t*G*128 + p*G + j.
    """
    gr = G * 128
    tfull = rows // gr
    tail = rows - tfull * gr
    tail_p = tail // G
    conf = np.empty(rows, np.float32)
    if PACK_ARGMAX:
        aux_2d = 127 - (mt_2d.view(np.uint32) & np.uint32(0x7F))
    else:
        aux_2d = mt_2d
    aux = np.empty(rows, aux_2d.dtype)
    nmain = tfull * gr
    conf[:nmain] = (
        conf_2d[:, : tfull * G].reshape(128, tfull, G).transpose(1, 0, 2).reshape(-1)
    )
    aux[:nmain] = (
        aux_2d[:, : tfull * G].reshape(128, tfull, G).transpose(1, 0, 2).reshape(-1)
    )
    if tail:
        conf[nmain:] = conf_2d[:tail_p, tfull * G :].reshape(-1)
        aux[nmain:] = aux_2d[:tail_p, tfull * G :].reshape(-1)
    return conf, aux


def _finish(conf, acc):
    """Mirror of the reference ECE finishing on host."""
    n = conf.shape[0]
    sorted_conf = np.sort(conf)
    q = np.linspace(0.0, float(n), NBINS + 1, dtype=np.float32)
    edges = np.interp(q, np.arange(n, dtype=np.float32), sorted_conf).astype(np.float32)
    idx = np.searchsorted(edges[1:-1], conf, side="left")
    valid = (conf > edges[0]) & (conf <= edges[-1])
    idx = np.where(valid, idx, NBINS)
    cnt = np.bincount(idx, minlength=NBINS + 1)[:NBINS].astype(np.float32)
    csum = np.bincount(idx, weights=conf.astype(np.float64), minlength=NBINS + 1)[
        :NBINS
    ].astype(np.float32)
    asum = np.bincount(idx, weights=acc.astype(np.float64), minlength=NBINS + 1)[
        :NBINS
    ].astype(np.float32)
    prop = cnt / np.float32(n)
    safe = np.maximum(cnt, 1.0)
    gap = np.abs(csum / safe - asum / safe)
    ece = np.sum(np.where(cnt > 0, gap * prop, 0.0), dtype=np.float32)
    return np.asarray(ece, dtype=np.float32).reshape(1)


def kernel(logits, labels, trace: bool = False):
    global LAST_RESULT
    logits = np.asarray(logits)
    labels = np.asarray(labels)
    assert logits.shape == (N, C), logits.shape

    if "nc" not in _CACHE:
        _CACHE["nc"] = _build(ROWS)
    nc = _CACHE["nc"]

    in_maps = [
        {"logits": np.ascontiguousarray(logits[i * ROWS : (i + 1) * ROWS], np.float32)}
        for i in range(NCORES)
    ]
    res = run_bass_kernel_spmd(nc, in_maps, core_ids=list(range(NCORES)), trace=trace)
    LAST_RESULT = res

    conf = np.empty(N, np.float32)
    aux = None
    for i in range(NCORES):
        c_i, a_i = _decode(res.results[i]["conf"], res.results[i]["mt"], ROWS)
        if aux is None:
            aux = np.empty(N, a_i.dtype)
        conf[i * ROWS : (i + 1) * ROWS] = c_i
        aux[i * ROWS : (i + 1) * ROWS] = a_i

    if PACK_ARGMAX:
        acc = (aux.astype(np.int64) == labels.astype(np.int64)).astype(np.float32)
    else:
        # aux = exact per-row max (f32); accuracy = logit at the label equals it
        xlab = logits[np.arange(N), labels.astype(np.int64)]
        acc = (xlab == aux).astype(np.float32)
    return _finish(conf, acc)
